# revision 18
# baseline (speedup 1.0000x reference)
"""Multi-head causal attention on 8 Trainium2 NeuronCores.

Sharding: core c handles batch b=c//4, head group g=c%4 (4 heads of 16).
Per-core Bass kernel computes QKV projection, causal attention in a
transposed-scores layout, and the out-projection partial; the host sums
the 4 per-batch bf16 partials (the out_proj all-reduce) in fp32 + bias.

v2 layout notes (per core, S=2048 tokens, D=1024, 4 heads x dh=64):
  - qt/kt [128, pair, S] bf16: partitions 0:64 = even head dh, 64:128 =
    odd head dh. No zero padding.
  - scores: the two heads of a pair run as CONCURRENT row-tiled K=64
    matmuls (tile_position (0,0) and (64,0)) into two PSUM banks of one
    [128, 1024] tile -> ~2x score throughput vs serial K=128.
  - exp: one ACT per k-chunk over both heads' banks ([128, 2, 512-q0]).
  - ctx: per head one matmul accumulating over k-chunks. A ones-column
    is folded into the V stationary operand so the softmax DENOMINATOR
    accumulates in the same PSUM bank for free (no DVE row-sum chain):
      even head: lhsT = [v(64) | 1] (M=65)  -> ctx at rows 0:64, den row 64
      odd head:  lhsT = [0(32)|1|0(31)|v(64)] (M=128) -> den row 32,
                 ctx at rows 64:128 (lane-aligned with ctxT's B half).
  - denominators: DVE reciprocal_approx_fast on the den rows, then K=1
    ones-matmuls (bf16 x f32r) replicate 1/den across 64 partitions;
    ctxT (staged raw in bf16) is normalized in place by two DVE muls.
  - out^T partial [D, S] bf16 = wo.T @ ctxT, accumulated over the 2
    pairs; host sums partials in fp32.
"""

import sys

sys.path.insert(0, "/opt/trn_rl_repo")

import numpy as np
import ml_dtypes

import concourse.bass as bass
import concourse.tile as tile
from concourse import bacc, mybir
from concourse import bass_utils

BF16 = ml_dtypes.bfloat16
F32 = mybir.dt.float32
F32R = mybir.dt.float32r
BF = mybir.dt.bfloat16

N_CORES = 8
S = 2048          # tokens
D = 1024          # model dim
DHC = 256         # head dims per core (4 heads x 64)
DH = 64
NQ = 4            # q chunks of 512
NK = 16           # k chunks of 128
NO = 8            # d_in / d_out chunks of 128

_NC_CACHE = None


def _build_core_kernel():
    nc = bacc.Bacc("TRN2", target_bir_lowering=False, debug=False,
                   num_devices=N_CORES)
    xT = nc.dram_tensor("xT", [D, S], BF, kind="ExternalInput").ap()
    w_all = nc.dram_tensor("w_all", [D, 3 * DHC], BF, kind="ExternalInput").ap()
    wo = nc.dram_tensor("wo", [DHC, D], BF, kind="ExternalInput").ap()
    masks = nc.dram_tensor("masks", [128, 128], BF, kind="ExternalInput").ap()
    outT = nc.dram_tensor("outT", [D, S], BF, kind="ExternalOutput").ap()

    with tile.TileContext(nc) as tc:
        _emit(tc, xT, w_all, wo, masks, outT)
    nc.compile()
    return nc


def _emit(tc, xT, w_all, wo, masks, outT):
    nc = tc.nc
    EXPF = mybir.ActivationFunctionType.Exp

    from contextlib import ExitStack
    ctx = ExitStack()
    const = ctx.enter_context(tc.tile_pool(name="const", bufs=1))
    work = ctx.enter_context(tc.tile_pool(name="work", bufs=3))
    recp = ctx.enter_context(tc.tile_pool(name="recp", bufs=2))
    outp = ctx.enter_context(tc.tile_pool(name="outp", bufs=3))
    ps_s = ctx.enter_context(tc.tile_pool(name="ps_s", bufs=2, space="PSUM"))
    ps_c = ctx.enter_context(tc.tile_pool(name="ps_c", bufs=2, space="PSUM"))
    ps_x = ctx.enter_context(tc.tile_pool(name="ps_x", bufs=2, space="PSUM"))

    # ---- persistent SBUF tensors ----
    xt = const.tile([128, NO, S], BF, tag="xt")          # x^T, d_in chunks
    wa = const.tile([128, NO, 3 * DHC], BF, tag="wa")    # [Wq|Wk|Wv] slices
    wos = const.tile([128, 2, D], BF, tag="wos")         # Wo row chunks
    msk = const.tile([128, 128], BF, tag="msk")          # causal staircase
    qt = const.tile([128, 2, S], BF, tag="qt")           # q^T per pair
    kt = const.tile([128, 2, S], BF, tag="kt")           # k^T per pair
    # v + folded ones columns (see module docstring)
    vsb = const.tile([128, NK, 4, 128], BF, tag="vsb")
    ctxT = const.tile([128, 2, S], BF, tag="ctxT")
    ones = const.tile([128, DH], BF, tag="ones")

    nc.sync.dma_start(msk[:], masks)
    nc.sync.dma_start(wos[:], wo.rearrange("(c p) f -> p c f", p=128))
    # interleaved per-chunk DMAs so the streaming matmuls start after the
    # first (wa, xt) chunk pair lands instead of after the full weights.
    wao = w_all.rearrange("(o p) f -> o p f", p=128)
    xTo = xT.rearrange("(o p) s -> o p s", p=128)
    for o in range(NO):
        nc.sync.dma_start(wa[:, o, :], wao[o])
        nc.sync.dma_start(xt[:, o, :], xTo[o])
    nc.vector.memset(ones[:], 1.0)
    # odd-head slots: zero cols 0:64, ones col 32 (denominator row source);
    # even-head slots: ones col 64. Cols 65:128 of even slots stay garbage
    # (never read: even lhsT slice is [:, 0:65]).
    nc.vector.memset(vsb[:, :, 1::2, 0:DH], 0.0)
    nc.vector.memset(vsb[:, :, 0::2, DH], 1.0)
    nc.vector.memset(vsb[:, :, 1::2, 32], 1.0)

    # ---- QKV projections ----
    def emit_qk(m, streaming=False):
        # qkvT chunk m: [128 dims, S] = w_all[:, m-slice].T @ x^T
        # streaming=True: o-outer loop so work starts as x^T chunks land.
        if streaming:
            pq0 = ps_s.tile([128, 1024], F32, tag="ps")
            pq1 = ps_s.tile([128, 1024], F32, tag="ps")
            pqs = [pq0, pq1]
            for o in range(NO):
                for n in range(NQ):
                    nc.tensor.matmul(
                        pqs[n // 2][:, 512 * (n % 2):512 * (n % 2) + 512],
                        lhsT=wa[:, o, 128 * m:128 * m + 128],
                        rhs=xt[:, o, 512 * n:512 * n + 512],
                        start=(o == 0), stop=(o == NO - 1),
                        skip_group_check=True)
        for n in range(NQ):
            n_sl = slice(512 * n, 512 * n + 512)
            if streaming:
                pq = pqs[n // 2][:, 512 * (n % 2):512 * (n % 2) + 512]
            else:
                pq = ps_x.tile([128, 512], F32, tag="px")
                for o in range(NO):
                    nc.tensor.matmul(
                        pq[:], lhsT=wa[:, o, 128 * m:128 * m + 128],
                        rhs=xt[:, o, n_sl],
                        start=(o == 0), stop=(o == NO - 1))
            if m < 2:
                nc.vector.tensor_copy(qt[:, m, n_sl], pq[:])
            else:
                nc.vector.tensor_copy(kt[:, m - 2, n_sl], pq[:])

    def emit_v():
        # v^T chunks (wa stationary, reused across 4 n-blocks -> 8x fewer
        # LDWEIGHTS than token-stationary), then DMA-xbar transpose into
        # the token-major vsb layout.
        vTs = const.tile([128, 2, S], BF, tag="vTs")
        for c in range(2):
            pv0 = ps_s.tile([128, 1024], F32, tag="ps")
            pv1 = ps_s.tile([128, 1024], F32, tag="ps")
            pvs = [pv0, pv1]
            c_sl = slice(2 * DHC + 128 * c, 2 * DHC + 128 * c + 128)
            for o in range(NO):
                for n in range(NQ):
                    nc.tensor.matmul(
                        pvs[n // 2][:, 512 * (n % 2):512 * (n % 2) + 512],
                        lhsT=wa[:, o, c_sl],
                        rhs=xt[:, o, 512 * n:512 * n + 512],
                        start=(o == 0), stop=(o == NO - 1),
                        skip_group_check=True)
            for n in range(NQ):
                nc.vector.tensor_copy(
                    vTs[:, c, 512 * n:512 * n + 512],
                    pvs[n // 2][:, 512 * (n % 2):512 * (n % 2) + 512])
        for t in range(NK):
            t_sl = slice(128 * t, 128 * t + 128)
            for c in range(2):
                nc.sync.dma_start_transpose(vsb[:, t, 2 * c, 0:DH],
                                            vTs[0:64, c, t_sl])
                nc.sync.dma_start_transpose(vsb[:, t, 2 * c + 1, DH:128],
                                            vTs[64:128, c, t_sl])

    # ---- attention for one (pair, q-window) ----
    def make_qk_filler(ms):
        # generator of single-instruction steps computing qkvT chunks for
        # the given m-chunks; spliced into attention windows as PE filler.
        for m in ms:
            for n in range(NQ):
                n_sl = slice(512 * n, 512 * n + 512)
                pq = ps_x.tile([128, 512], F32, tag="px")
                for o in range(NO):
                    nc.tensor.matmul(
                        pq[:], lhsT=wa[:, o, 128 * m:128 * m + 128],
                        rhs=xt[:, o, n_sl],
                        start=(o == 0), stop=(o == NO - 1),
                        skip_group_check=True)
                    yield
                if m < 2:
                    nc.vector.tensor_copy(qt[:, m, n_sl], pq[:])
                else:
                    nc.vector.tensor_copy(kt[:, m - 2, n_sl], pq[:])
                yield

    def emit_attn_chunk(p, j, filler=None, per_i=0, pending_tail=None,
                        cast_eng="v"):
        n_i = 4 * j + 4
        q_sl = slice(512 * j, 512 * j + 512)
        pcA = ps_c.tile([128, 512], F32, tag="pc")
        pcB = ps_c.tile([128, 512], F32, tag="pc")

        def emit_scores(i):
            d = i - 4 * j
            # diagonal tiles: k-chunk i only reaches q >= 128*d in this
            # q-window; restrict all work to the valid column range.
            q0 = 128 * d if d > 0 else 0
            k_sl = slice(128 * i, 128 * i + 128)
            qv_sl = slice(512 * j + q0, 512 * j + 512)
            pss = ps_s.tile([128, 1024], F32, tag="ps")
            # two heads as concurrent row-tiled K=64 matmuls
            nc.tensor.matmul(pss[:, q0:512],
                             lhsT=kt[0:64, p, k_sl], rhs=qt[0:64, p, qv_sl],
                             start=True, stop=True)
            nc.tensor.matmul(pss[:, 512 + q0:1024],
                             lhsT=kt[64:128, p, k_sl],
                             rhs=qt[64:128, p, qv_sl],
                             start=True, stop=True)
            return pss, q0

        # software pipeline: scores for i+1 are emitted before ctx of i so
        # the PE never sits behind a wait on the exp of i.
        pss_cur, q0_cur = emit_scores(0)
        if pending_tail is not None:
            pending_tail()
        for i in range(n_i):
            q0 = q0_cur
            eT = work.tile([128, 2, 512], BF, tag="exp")
            pv2 = pss_cur.rearrange("p (g f) -> p g f", g=2)
            nc.scalar.activation(eT[:, :, q0:512], pv2[:, :, q0:512],
                                 EXPF, scale=0.125)
            if i + 1 < n_i:
                pss_cur, q0_cur = emit_scores(i + 1)
            if filler is not None:
                for _ in range(per_i):
                    next(filler, None)
            if i - 4 * j >= 0:  # triangular 128x128 mask on the diagonal
                for h in (0, 1):
                    nc.vector.tensor_mul(eT[:, h, q0:q0 + 128],
                                         eT[:, h, q0:q0 + 128], msk[:])
            # ctx accumulation; ones columns accumulate denominators
            nc.tensor.matmul(
                pcA[0:65, q0:512], lhsT=vsb[:, i, 2 * p, 0:65],
                rhs=eT[:, 0, q0:512],
                start=(i == 0), stop=(i == n_i - 1), skip_group_check=True)
            nc.tensor.matmul(
                pcB[:, q0:512], lhsT=vsb[:, i, 2 * p + 1, :],
                rhs=eT[:, 1, q0:512],
                start=(i == 0), stop=(i == n_i - 1), skip_group_check=True)
        # ---- window tail: reciprocal, raw evacuation, normalize ----
        # window tail, returned as a closure so the caller can emit it AFTER
        # the next window's first scores (keeps the PE stream dense):
        # stage denominators to SBUF (bf16), replicate across the head's 64
        # partitions with K=1 ones-matmuls, then one base-0 reciprocal.
        # (reciprocal_approx_fast/partition_broadcast silently misbehave on
        # HW at base partition != 0, so the recip must run from partition 0.)
        def tail():
            cp = nc.vector.tensor_copy if cast_eng == "v" else nc.scalar.copy
            den = recp.tile([128, 512], BF, tag="den")
            rec = recp.tile([128, 512], F32, tag="rec")
            cp(den[64:65, :], pcA[64:65, :])
            cp(ctxT[0:64, p, q_sl], pcA[0:64, :])
            cp(den[32:33, :], pcB[32:33, :])
            cp(ctxT[64:128, p, q_sl], pcB[64:128, :])
            pd = ps_x.tile([128, 512], F32, tag="px")
            nc.tensor.matmul(pd[0:64, :], lhsT=ones[64:65, :],
                             rhs=den[64:65, :],
                             start=True, stop=True, tile_position=(64, 0))
            nc.tensor.matmul(pd[64:128, :], lhsT=ones[32:33, :],
                             rhs=den[32:33, :],
                             start=True, stop=True, tile_position=(32, 64))
            nc.vector.reciprocal_approx_fast(out=rec[:, :], in_=pd[:, :])
            nc.vector.tensor_mul(ctxT[0:64, p, q_sl], ctxT[0:64, p, q_sl],
                                 rec[0:64, :])
            nc.vector.tensor_mul(ctxT[64:128, p, q_sl],
                                 ctxT[64:128, p, q_sl], rec[64:128, :])
        return tail

    # ---- out projection for one token block: outT[:, n] += wo.T @ ctxT ----
    outT_m = outT.rearrange("(mm p) s -> mm p s", p=128)

    def op_steps(n, eng="v"):
        n_sl = slice(512 * n, 512 * n + 512)
        for m in range(NO):
            po = ps_x.tile([128, 512], F32, tag="px")
            for p in (0, 1):
                nc.tensor.matmul(
                    po[:], lhsT=wos[:, p, 128 * m:128 * m + 128],
                    rhs=ctxT[:, p, n_sl],
                    start=(p == 0), stop=(p == 1), skip_group_check=True)
                yield
            osb = outp.tile([128, 512], BF, tag="osb")
            if eng == "v":
                nc.vector.tensor_copy(osb[:], po[:])
            else:
                nc.scalar.copy(osb[:], po[:])
            nc.sync.dma_start(outT_m[m, :, n_sl], osb[:])
            yield

    def emit_outproj_n(n, eng="v"):
        for _ in op_steps(n, eng):
            pass

    emit_qk(0, streaming=True)
    emit_qk(2, streaming=True)
    emit_v()
    # pair-1 QKV projections are spliced into the first two (pair-0)
    # windows as PE filler under their ACT-bound stretches.
    from itertools import chain
    fill = make_qk_filler([1, 3])
    t = emit_attn_chunk(0, 3, filler=fill, per_i=3)
    t = emit_attn_chunk(0, 2, filler=fill, per_i=3, pending_tail=t)
    for _ in fill:  # drain any remaining filler steps
        pass
    t = emit_attn_chunk(1, 3, pending_tail=t)
    t = emit_attn_chunk(1, 2, pending_tail=t)   # emits (1,3)'s tail
    # outproj for j=3/2 spliced into the last four windows; ctxT for window
    # j is complete once (1,j)'s tail has been emitted (pending_tail at the
    # START of the following window, before any filler step runs).
    fill2 = chain(op_steps(3), op_steps(2))
    t = emit_attn_chunk(0, 1, filler=fill2, per_i=3, pending_tail=t)
    t = emit_attn_chunk(0, 0, filler=fill2, per_i=3, pending_tail=t)
    t = emit_attn_chunk(1, 1, filler=fill2, per_i=3, pending_tail=t)
    for _ in fill2:
        pass
    t = emit_attn_chunk(1, 0, pending_tail=t, cast_eng="s")
    emit_outproj_n(1, eng="s")  # (1,1)'s tail already emitted above
    t()                         # (1,0)'s tail (scalar casts: ACT idle now)
    emit_outproj_n(0, eng="s")
    ctx.close()


def _get_nc():
    global _NC_CACHE
    if _NC_CACHE is None:
        _NC_CACHE = _build_core_kernel()
    return _NC_CACHE


def _build_masks():
    p = np.arange(128)[:, None]
    f = np.arange(128)[None, :]
    return (p <= f).astype(BF16)


def _shard_inputs(x, Wq, Wk, Wv, Wo):
    xb = x.astype(BF16)
    masks = _build_masks()
    in_maps = []
    for c in range(N_CORES):
        b, g = divmod(c, 4)
        cols = slice(DHC * g, DHC * g + DHC)
        w_all = np.ascontiguousarray(np.concatenate(
            [Wq[:, cols], Wk[:, cols], Wv[:, cols]], axis=1).astype(BF16))
        wo_s = np.ascontiguousarray(Wo[cols, :].astype(BF16))
        xT = np.ascontiguousarray(xb[b].T)
        in_maps.append({"xT": xT, "w_all": w_all, "wo": wo_s, "masks": masks})
    return in_maps


def _unshard(results, bo):
    out = np.empty((2, S, D), np.float32)
    for b in range(2):
        acc = results[4 * b]["outT"].astype(np.float32)
        for g in range(1, 4):
            acc += results[4 * b + g]["outT"].astype(np.float32)
        out[b] = acc.T + bo.astype(np.float32)
    return out


def run(x, Wq, Wk, Wv, Wo, bo, trace=False, **spmd_kwargs):
    nc = _get_nc()
    in_maps = _shard_inputs(x, Wq, Wk, Wv, Wo)
    res = bass_utils.run_bass_kernel_spmd(
        nc, in_maps, core_ids=list(range(N_CORES)), trace=trace,
        **spmd_kwargs)
    return _unshard(res.results, bo), res


def kernel(x, Wq, Wk, Wv, Wo, bo):
    out, _ = run(np.asarray(x), np.asarray(Wq), np.asarray(Wk),
                 np.asarray(Wv), np.asarray(Wo), np.asarray(bo))
    return out


# revision 19
# speedup vs baseline: 1.1822x; 1.1822x over previous
"""Multi-head causal attention on 8 Trainium2 NeuronCores.

Sharding: core c handles batch b=c//4, head group g=c%4 (4 heads of 16).
Per-core Bass kernel computes QKV projection, causal attention in a
transposed-scores layout, and the out-projection partial; the host sums
the 4 per-batch bf16 partials (the out_proj all-reduce) in fp32 + bias.

v2 layout notes (per core, S=2048 tokens, D=1024, 4 heads x dh=64):
  - qt/kt [128, pair, S] bf16: partitions 0:64 = even head dh, 64:128 =
    odd head dh. No zero padding.
  - scores: the two heads of a pair run as CONCURRENT row-tiled K=64
    matmuls (tile_position (0,0) and (64,0)) into two PSUM banks of one
    [128, 1024] tile -> ~2x score throughput vs serial K=128.
  - exp: one ACT per k-chunk over both heads' banks ([128, 2, 512-q0]).
  - ctx: per head one matmul accumulating over k-chunks. A ones-column
    is folded into the V stationary operand so the softmax DENOMINATOR
    accumulates in the same PSUM bank for free (no DVE row-sum chain):
      even head: lhsT = [v(64) | 1] (M=65)  -> ctx at rows 0:64, den row 64
      odd head:  lhsT = [0(32)|1|0(31)|v(64)] (M=128) -> den row 32,
                 ctx at rows 64:128 (lane-aligned with ctxT's B half).
  - denominators: DVE reciprocal_approx_fast on the den rows, then K=1
    ones-matmuls (bf16 x f32r) replicate 1/den across 64 partitions;
    ctxT (staged raw in bf16) is normalized in place by two DVE muls.
  - out^T partial [D, S] bf16 = wo.T @ ctxT, accumulated over the 2
    pairs; host sums partials in fp32.
"""

import sys

sys.path.insert(0, "/opt/trn_rl_repo")

import numpy as np
import ml_dtypes

import concourse.bass as bass
import concourse.tile as tile
from concourse import bacc, mybir
from concourse import bass_utils

BF16 = ml_dtypes.bfloat16
F32 = mybir.dt.float32
F32R = mybir.dt.float32r
BF = mybir.dt.bfloat16

N_CORES = 8
S = 2048          # tokens
D = 1024          # model dim
DHC = 256         # head dims per core (4 heads x 64)
DH = 64
NQ = 4            # q chunks of 512
NK = 16           # k chunks of 128
NO = 8            # d_in / d_out chunks of 128

_NC_CACHE = None


def _build_core_kernel():
    nc = bacc.Bacc("TRN2", target_bir_lowering=False, debug=False,
                   num_devices=N_CORES)
    xT = nc.dram_tensor("xT", [D, S], BF, kind="ExternalInput").ap()
    w_all = nc.dram_tensor("w_all", [D, 3 * DHC], BF, kind="ExternalInput").ap()
    wo = nc.dram_tensor("wo", [DHC, D], BF, kind="ExternalInput").ap()
    masks = nc.dram_tensor("masks", [128, 128], BF, kind="ExternalInput").ap()
    outT = nc.dram_tensor("outT", [D, S], BF, kind="ExternalOutput").ap()

    with tile.TileContext(nc) as tc:
        _emit(tc, xT, w_all, wo, masks, outT)
    nc.compile()
    return nc


def _emit(tc, xT, w_all, wo, masks, outT):
    nc = tc.nc
    EXPF = mybir.ActivationFunctionType.Exp

    from contextlib import ExitStack
    ctx = ExitStack()
    const = ctx.enter_context(tc.tile_pool(name="const", bufs=1))
    work = ctx.enter_context(tc.tile_pool(name="work", bufs=3))
    recp = ctx.enter_context(tc.tile_pool(name="recp", bufs=2))
    outp = ctx.enter_context(tc.tile_pool(name="outp", bufs=3))
    ps_s = ctx.enter_context(tc.tile_pool(name="ps_s", bufs=2, space="PSUM"))
    ps_c = ctx.enter_context(tc.tile_pool(name="ps_c", bufs=2, space="PSUM"))
    ps_x = ctx.enter_context(tc.tile_pool(name="ps_x", bufs=2, space="PSUM"))

    # ---- persistent SBUF tensors ----
    xt = const.tile([128, NO, S], BF, tag="xt")          # x^T, d_in chunks
    wa = const.tile([128, NO, 3 * DHC], BF, tag="wa")    # [Wq|Wk|Wv] slices
    wos = const.tile([128, 2, D], BF, tag="wos")         # Wo row chunks
    msk = const.tile([128, 128], BF, tag="msk")          # causal staircase
    qt = const.tile([128, 2, S], BF, tag="qt")           # q^T per pair
    kt = const.tile([128, 2, S], BF, tag="kt")           # k^T per pair
    # v + folded ones columns (see module docstring)
    vsb = const.tile([128, NK, 4, 128], BF, tag="vsb")
    ctxT = const.tile([128, 2, S], BF, tag="ctxT")
    ones = const.tile([128, DH], BF, tag="ones")

    nc.sync.dma_start(msk[:], masks)
    nc.sync.dma_start(wos[:], wo.rearrange("(c p) f -> p c f", p=128))
    # interleaved per-chunk DMAs so the streaming matmuls start after the
    # first (wa, xt) chunk pair lands instead of after the full weights.
    wao = w_all.rearrange("(o p) f -> o p f", p=128)
    xTo = xT.rearrange("(o p) s -> o p s", p=128)
    for o in range(NO):
        nc.sync.dma_start(wa[:, o, :], wao[o])
        nc.sync.dma_start(xt[:, o, :], xTo[o])
    nc.vector.memset(ones[:], 1.0)
    # odd-head slots: zero cols 0:64, ones col 32 (denominator row source);
    # even-head slots: ones col 64. Cols 65:128 of even slots stay garbage
    # (never read: even lhsT slice is [:, 0:65]).
    nc.vector.memset(vsb[:, :, 1::2, 0:DH], 0.0)
    nc.vector.memset(vsb[:, :, 0::2, DH], 1.0)
    nc.vector.memset(vsb[:, :, 1::2, 32], 1.0)

    # ---- QKV projections ----
    def emit_qk(m, streaming=False):
        # qkvT chunk m: [128 dims, S] = w_all[:, m-slice].T @ x^T
        # streaming=True: o-outer loop so work starts as x^T chunks land.
        if streaming:
            pq0 = ps_s.tile([128, 1024], F32, tag="ps")
            pq1 = ps_s.tile([128, 1024], F32, tag="ps")
            pqs = [pq0, pq1]
            for o in range(NO):
                for n in range(NQ):
                    nc.tensor.matmul(
                        pqs[n // 2][:, 512 * (n % 2):512 * (n % 2) + 512],
                        lhsT=wa[:, o, 128 * m:128 * m + 128],
                        rhs=xt[:, o, 512 * n:512 * n + 512],
                        start=(o == 0), stop=(o == NO - 1),
                        skip_group_check=True)
        for n in range(NQ):
            n_sl = slice(512 * n, 512 * n + 512)
            if streaming:
                pq = pqs[n // 2][:, 512 * (n % 2):512 * (n % 2) + 512]
            else:
                pq = ps_x.tile([128, 512], F32, tag="px")
                for o in range(NO):
                    nc.tensor.matmul(
                        pq[:], lhsT=wa[:, o, 128 * m:128 * m + 128],
                        rhs=xt[:, o, n_sl],
                        start=(o == 0), stop=(o == NO - 1))
            if m < 2:
                nc.vector.tensor_copy(qt[:, m, n_sl], pq[:])
            else:
                nc.vector.tensor_copy(kt[:, m - 2, n_sl], pq[:])

    def emit_v():
        # v [tokens, 4*dh] = x @ Wv  (x^T chunks are the stationary side)
        for t in range(NK):
            pv = ps_x.tile([128, 512], F32, tag="px")
            for o in range(NO):
                nc.tensor.matmul(
                    pv[:, :DHC], lhsT=xt[:, o, 128 * t:128 * t + 128],
                    rhs=wa[:, o, 2 * DHC:3 * DHC],
                    start=(o == 0), stop=(o == NO - 1))
            pv4 = pv[:, :DHC].rearrange("p (h c) -> p h c", c=DH)
            # even heads -> cols 0:64, odd heads -> cols 64:128
            nc.vector.tensor_copy(vsb[:, t, 0::2, 0:DH], pv4[:, 0::2, :])
            nc.vector.tensor_copy(vsb[:, t, 1::2, DH:128], pv4[:, 1::2, :])

    # ---- attention for one (pair, q-window) ----
    def make_qk_filler(ms):
        # generator of single-instruction steps computing qkvT chunks for
        # the given m-chunks; spliced into attention windows as PE filler.
        for m in ms:
            for n in range(NQ):
                n_sl = slice(512 * n, 512 * n + 512)
                pq = ps_x.tile([128, 512], F32, tag="px")
                for o in range(NO):
                    nc.tensor.matmul(
                        pq[:], lhsT=wa[:, o, 128 * m:128 * m + 128],
                        rhs=xt[:, o, n_sl],
                        start=(o == 0), stop=(o == NO - 1),
                        skip_group_check=True)
                    yield
                if m < 2:
                    nc.vector.tensor_copy(qt[:, m, n_sl], pq[:])
                else:
                    nc.vector.tensor_copy(kt[:, m - 2, n_sl], pq[:])
                yield

    def emit_attn_chunk(p, j, filler=None, per_i=0, pending_tail=None,
                        cast_eng="v"):
        n_i = 4 * j + 4
        q_sl = slice(512 * j, 512 * j + 512)
        pcA = ps_c.tile([128, 512], F32, tag="pc")
        pcB = ps_c.tile([128, 512], F32, tag="pc")

        def emit_scores(i):
            d = i - 4 * j
            # diagonal tiles: k-chunk i only reaches q >= 128*d in this
            # q-window; restrict all work to the valid column range.
            q0 = 128 * d if d > 0 else 0
            k_sl = slice(128 * i, 128 * i + 128)
            qv_sl = slice(512 * j + q0, 512 * j + 512)
            pss = ps_s.tile([128, 1024], F32, tag="ps")
            # two heads as concurrent row-tiled K=64 matmuls
            nc.tensor.matmul(pss[:, q0:512],
                             lhsT=kt[0:64, p, k_sl], rhs=qt[0:64, p, qv_sl],
                             start=True, stop=True)
            nc.tensor.matmul(pss[:, 512 + q0:1024],
                             lhsT=kt[64:128, p, k_sl],
                             rhs=qt[64:128, p, qv_sl],
                             start=True, stop=True)
            return pss, q0

        # software pipeline: scores for i+1 are emitted before ctx of i so
        # the PE never sits behind a wait on the exp of i.
        pss_cur, q0_cur = emit_scores(0)
        if pending_tail is not None:
            pending_tail()
        for i in range(n_i):
            q0 = q0_cur
            eT = work.tile([128, 2, 512], BF, tag="exp")
            pv2 = pss_cur.rearrange("p (g f) -> p g f", g=2)
            nc.scalar.activation(eT[:, :, q0:512], pv2[:, :, q0:512],
                                 EXPF, scale=0.125)
            if i + 1 < n_i:
                pss_cur, q0_cur = emit_scores(i + 1)
            if filler is not None:
                for _ in range(per_i):
                    next(filler, None)
            if i - 4 * j >= 0:  # triangular 128x128 mask on the diagonal
                for h in (0, 1):
                    nc.vector.tensor_mul(eT[:, h, q0:q0 + 128],
                                         eT[:, h, q0:q0 + 128], msk[:])
            # ctx accumulation; ones columns accumulate denominators
            nc.tensor.matmul(
                pcA[0:65, q0:512], lhsT=vsb[:, i, 2 * p, 0:65],
                rhs=eT[:, 0, q0:512],
                start=(i == 0), stop=(i == n_i - 1), skip_group_check=True)
            nc.tensor.matmul(
                pcB[:, q0:512], lhsT=vsb[:, i, 2 * p + 1, :],
                rhs=eT[:, 1, q0:512],
                start=(i == 0), stop=(i == n_i - 1), skip_group_check=True)
        # ---- window tail: reciprocal, raw evacuation, normalize ----
        # window tail, returned as a closure so the caller can emit it AFTER
        # the next window's first scores (keeps the PE stream dense):
        # stage denominators to SBUF (bf16), replicate across the head's 64
        # partitions with K=1 ones-matmuls, then one base-0 reciprocal.
        # (reciprocal_approx_fast/partition_broadcast silently misbehave on
        # HW at base partition != 0, so the recip must run from partition 0.)
        def tail():
            cp = nc.vector.tensor_copy if cast_eng == "v" else nc.scalar.copy
            den = recp.tile([128, 512], BF, tag="den")
            rec = recp.tile([128, 512], F32, tag="rec")
            cp(den[64:65, :], pcA[64:65, :])
            cp(ctxT[0:64, p, q_sl], pcA[0:64, :])
            cp(den[32:33, :], pcB[32:33, :])
            cp(ctxT[64:128, p, q_sl], pcB[64:128, :])
            pd = ps_x.tile([128, 512], F32, tag="px")
            nc.tensor.matmul(pd[0:64, :], lhsT=ones[64:65, :],
                             rhs=den[64:65, :],
                             start=True, stop=True, tile_position=(64, 0))
            nc.tensor.matmul(pd[64:128, :], lhsT=ones[32:33, :],
                             rhs=den[32:33, :],
                             start=True, stop=True, tile_position=(32, 64))
            nc.vector.reciprocal_approx_fast(out=rec[:, :], in_=pd[:, :])
            nc.vector.tensor_mul(ctxT[0:64, p, q_sl], ctxT[0:64, p, q_sl],
                                 rec[0:64, :])
            nc.vector.tensor_mul(ctxT[64:128, p, q_sl],
                                 ctxT[64:128, p, q_sl], rec[64:128, :])
        return tail

    # ---- out projection for one token block: outT[:, n] += wo.T @ ctxT ----
    outT_m = outT.rearrange("(mm p) s -> mm p s", p=128)

    def op_steps(n, eng="v"):
        n_sl = slice(512 * n, 512 * n + 512)
        for m in range(NO):
            po = ps_x.tile([128, 512], F32, tag="px")
            for p in (0, 1):
                nc.tensor.matmul(
                    po[:], lhsT=wos[:, p, 128 * m:128 * m + 128],
                    rhs=ctxT[:, p, n_sl],
                    start=(p == 0), stop=(p == 1), skip_group_check=True)
                yield
            osb = outp.tile([128, 512], BF, tag="osb")
            if eng == "v":
                nc.vector.tensor_copy(osb[:], po[:])
            else:
                nc.scalar.copy(osb[:], po[:])
            nc.sync.dma_start(outT_m[m, :, n_sl], osb[:])
            yield

    def emit_outproj_n(n, eng="v"):
        for _ in op_steps(n, eng):
            pass

    emit_qk(0, streaming=True)
    emit_qk(2, streaming=True)
    emit_v()
    # pair-1 QKV projections are spliced into the first two (pair-0)
    # windows as PE filler under their ACT-bound stretches.
    from itertools import chain
    fill = make_qk_filler([1, 3])
    t = emit_attn_chunk(0, 3, filler=fill, per_i=3)
    t = emit_attn_chunk(0, 2, filler=fill, per_i=3, pending_tail=t)
    for _ in fill:  # drain any remaining filler steps
        pass
    t = emit_attn_chunk(1, 3, pending_tail=t)
    t = emit_attn_chunk(1, 2, pending_tail=t)   # emits (1,3)'s tail
    # outproj for j=3/2 spliced into the last four windows; ctxT for window
    # j is complete once (1,j)'s tail has been emitted (pending_tail at the
    # START of the following window, before any filler step runs).
    fill2 = chain(op_steps(3), op_steps(2))
    t = emit_attn_chunk(0, 1, filler=fill2, per_i=3, pending_tail=t)
    t = emit_attn_chunk(0, 0, filler=fill2, per_i=3, pending_tail=t)
    t = emit_attn_chunk(1, 1, filler=fill2, per_i=3, pending_tail=t)
    for _ in fill2:
        pass
    t = emit_attn_chunk(1, 0, pending_tail=t, cast_eng="s")
    emit_outproj_n(1, eng="s")  # (1,1)'s tail already emitted above
    t()                         # (1,0)'s tail (scalar casts: ACT idle now)
    emit_outproj_n(0, eng="s")
    ctx.close()


def _get_nc():
    global _NC_CACHE
    if _NC_CACHE is None:
        _NC_CACHE = _build_core_kernel()
    return _NC_CACHE


def _build_masks():
    p = np.arange(128)[:, None]
    f = np.arange(128)[None, :]
    return (p <= f).astype(BF16)


def _shard_inputs(x, Wq, Wk, Wv, Wo):
    xb = x.astype(BF16)
    masks = _build_masks()
    in_maps = []
    for c in range(N_CORES):
        b, g = divmod(c, 4)
        cols = slice(DHC * g, DHC * g + DHC)
        w_all = np.ascontiguousarray(np.concatenate(
            [Wq[:, cols], Wk[:, cols], Wv[:, cols]], axis=1).astype(BF16))
        wo_s = np.ascontiguousarray(Wo[cols, :].astype(BF16))
        xT = np.ascontiguousarray(xb[b].T)
        in_maps.append({"xT": xT, "w_all": w_all, "wo": wo_s, "masks": masks})
    return in_maps


def _unshard(results, bo):
    out = np.empty((2, S, D), np.float32)
    for b in range(2):
        acc = results[4 * b]["outT"].astype(np.float32)
        for g in range(1, 4):
            acc += results[4 * b + g]["outT"].astype(np.float32)
        out[b] = acc.T + bo.astype(np.float32)
    return out


def run(x, Wq, Wk, Wv, Wo, bo, trace=False, **spmd_kwargs):
    nc = _get_nc()
    in_maps = _shard_inputs(x, Wq, Wk, Wv, Wo)
    res = bass_utils.run_bass_kernel_spmd(
        nc, in_maps, core_ids=list(range(N_CORES)), trace=trace,
        **spmd_kwargs)
    return _unshard(res.results, bo), res


def kernel(x, Wq, Wk, Wv, Wo, bo):
    out, _ = run(np.asarray(x), np.asarray(Wq), np.asarray(Wk),
                 np.asarray(Wv), np.asarray(Wo), np.asarray(bo))
    return out
